# revision 4
# baseline (speedup 1.0000x reference)
"""CommNet forward kernel v2 for 8 Trainium2 NeuronCores.

Layout: feature-major, TWO rows (agents) per PE column via K=128
block-diagonal weights.  A unit is [128, 512]: partitions = 2 halves x
64 features (row half A = agents 0-15, half B = agents 16-31), columns
= 32 samples x 16 (16 adjacent columns = one sample's 32 agents split
across the halves).  Every dense layer is a K=128, M=128 block-diag
matmul charging 512 output columns per 1024 rows — half the per-row
cost of a K=64 layout.  512-col units keep each stage PSUM tile to one
bank (PSUM: 8 banks is the pipelining constraint).

DMA discipline: the cost model holds the issuing sequencer for the
whole descriptor-gen + transfer + completion-sem chain (~3 us per
dma_start), so everything is batched: one weights DMA, one biases DMA,
one pen DMA, obs in 8-unit chunks, q out every 8 units.

Per unit (512 cols, 32 samples):
  psE  = Wenc_bd  @ obs
  h0   = relu(psE + be)            [Act]
  round r in {0,1}:
    S2  = groupwise tensor_reduce(h) [DVE] -> [128, 32] bf16 in SBUF
    psR = Wself_bd[r] @ h  (+)  Wsum_vs[r] @ S2-broadcast (K=128 folds
          the cross-half agent sum into the matmul)
    h   = relu(psR + b_r)          [Pool]
  psH  = W1_bd @ h;  hid = relu(psH + bh)   [Act]
  out2: [32,512] into q-bank partition range 32*(u%4)
  every 4 units: q = psQ + pen (mask+bias folded host-side) [DVE]
"""

import contextlib
import sys

import numpy as np

sys.path.insert(0, "/opt/trn_rl_repo")

import ml_dtypes  # noqa: E402

B, N, OBS, H, A, NR = 16384, 32, 64, 64, 16, 2
NCORES = 8
RPC = B * N // NCORES      # rows per core = 65536

SUP = 512                  # columns per unit (= 1024 rows, 32 samples)
NSUP = RPC // (2 * SUP)    # 64 units
SAMP = SUP // 16           # samples per unit = 32
OCH = 8                    # units per obs DMA chunk
QCH = 8                    # units per q DMA chunk (2 q banks)

# weight buffer column offsets
WOFF = {"Wenc": 0, "Wself0": 128, "Wself1": 256, "W1": 384,
        "Wsum0": 512, "Wsum1": 640, "W2": 768, "idq": 800}
WCOLS = 928

_cache = {}


def _build_device_program():
    import concourse.bacc as bacc
    import concourse.mybir as mybir
    from concourse import tile

    F32 = mybir.dt.float32
    BF16 = mybir.dt.bfloat16
    Relu = mybir.ActivationFunctionType.Relu
    Copy = mybir.ActivationFunctionType.Copy
    ALU = mybir.AluOpType
    AXIS = mybir.AxisListType

    nc = bacc.Bacc("TRN2", target_bir_lowering=False, debug=False)

    obs_d = nc.dram_tensor("obs_pk", [NSUP // OCH, 128, OCH * SUP], BF16,
                           kind="ExternalInput")
    pen_d = nc.dram_tensor("pen_pk", [128, (NSUP // 4) * 512], BF16,
                           kind="ExternalInput")
    q_d = nc.dram_tensor("q_pk", [NSUP // QCH, 128, (QCH // 4) * 512], BF16,
                         kind="ExternalOutput")
    wall_d = nc.dram_tensor("Wall", [128, WCOLS], BF16, kind="ExternalInput")
    ball_d = nc.dram_tensor("Ball", [128, 4], F32, kind="ExternalInput")

    with tile.TileContext(nc) as tc, contextlib.ExitStack() as ctx:
        wp = ctx.enter_context(tc.tile_pool(name="w", bufs=1))
        pool = ctx.enter_context(tc.tile_pool(name="p", bufs=2))
        psum = ctx.enter_context(tc.tile_pool(name="ps", bufs=1, space="PSUM"))

        wall = wp.tile([128, WCOLS], BF16, tag="wall", name="wall")
        nc.sync.dma_start(wall[:], wall_d[:])
        ball = wp.tile([128, 4], F32, tag="ball", name="ball")
        nc.sync.dma_start(ball[:], ball_d[:])
        pen_t = wp.tile([128, (NSUP // 4) * 512], BF16, tag="pen", name="pen")
        pen_started = []

        def Wt(n):
            return wall[:, WOFF[n]:WOFF[n] + (32 if n == "W2" else 128)]

        def Bias(i):
            return ball[:, i:i + 1]

        # ---- software-pipelined emission across units -----------------
        # Segments per unit u (PE work + attached evacs), emitted with
        # unit-offsets so the PE stream interleaves ~4 units and every
        # matmul's inputs are evacuated well before the PE reaches it.
        st = {"obs": {}, "h0": {}, "h1": {}, "h2": {}, "hid": {},
              "S2": {}, "psR": {}, "psQ": {}, "qsb": {}}

        def seg_A(u):                      # enc
            if u % OCH == 0:
                obs_t = pool.tile([128, OCH * SUP], BF16, tag="obs", bufs=2, name=f"obs{u}")
                nc.sync.dma_start(obs_t[:], obs_d[u // OCH])
                st["obs"][u // OCH] = obs_t
            obs_t = st["obs"][u // OCH]
            osl = obs_t[:, (u % OCH) * SUP:(u % OCH + 1) * SUP]
            psE = psum.tile([128, SUP], F32, tag="stg", bufs=7, name=f"psE{u}")
            nc.tensor.matmul(psE[:], Wt("Wenc"), osl, start=True, stop=True)
            h = pool.tile([128, SUP], BF16, tag="h", bufs=24, name=f"h0_{u}")
            nc.scalar.activation(h[:], psE[:], Relu, bias=Bias(0))
            st["h0"][u] = h

        def seg_round_a(u, r):             # S-reduce + Wself
            h = st["h0" if r == 0 else "h1"][u]
            S2 = pool.tile([128, SAMP], BF16, tag="S2", bufs=6, name=f"S2_{u}_{r}")
            if r == 0:
                # Pool (gpsimd) cannot touch PSUM, but it can take this
                # all-SBUF pairwise-add tree off the DVE/Act critical path.
                tmp = pool.tile([128, 448], BF16, tag="rt", bufs=3,
                                name=f"rt{u}_{r}")
                src_ap, width = h, 512
                off = 0
                outs = [(tmp, 0, 256), (tmp, 256, 128), (tmp, 384, 64),
                        (S2, 0, 32)]
                prev = h[:]
                prev_off = 0
                for (dst, doff, w) in outs:
                    i3 = prev.rearrange("p (t e) -> p t e", e=2)
                    o3 = dst[:, doff:doff + w].rearrange("p (t o) -> p t o", o=1)
                    nc.gpsimd.tensor_tensor(o3, i3[:, :, 0:1], i3[:, :, 1:2],
                                            ALU.add)
                    prev = dst[:, doff:doff + w]
            else:
                with nc.allow_low_precision("bf16 sums feed 1/31-scaled msg"):
                    nc.vector.tensor_reduce(
                        S2[:], h[:].rearrange("p (t n) -> p t n", n=16),
                        AXIS.X, ALU.add)
            st["S2"][(u, r)] = S2
            psR = psum.tile([128, SUP], F32, tag="stg", bufs=7, name=f"psR{u}_{r}")
            nc.tensor.matmul(psR[:], Wt(f"Wself{r}"), h[:],
                             start=True, stop=False)
            st["psR"][(u, r)] = psR

        def seg_round_b(u, r):             # Wsum-broadcast + evac
            psR = st["psR"].pop((u, r))
            S2 = st["S2"].pop((u, r))
            rhs = S2[:].unsqueeze(2).broadcast_to([128, SAMP, 16])
            nc.tensor.matmul(
                psR[:].rearrange("p (t n) -> p t n", n=16),
                Wt(f"Wsum{r}"), rhs, start=False, stop=True)
            h = pool.tile([128, SUP], BF16, tag="h", bufs=24, name=f"h{1 + r}_{u}")
            if r == 1 and u % 3 != 2:
                nc.scalar.activation(h[:], psR[:], Relu, bias=Bias(2))
            else:
                nc.vector.tensor_scalar(h[:], psR[:], Bias(1 + r), 0.0,
                                        ALU.add, ALU.max)
            st["h1" if r == 0 else "h2"][u] = h

        def seg_D(u):                      # out1
            h = st["h2"].pop(u)
            psH = psum.tile([128, SUP], F32, tag="stg", bufs=7, name=f"psH{u}")
            nc.tensor.matmul(psH[:], Wt("W1"), h[:], start=True, stop=True)
            hid = pool.tile([128, SUP], BF16, tag="h", bufs=24, name=f"hid{u}")
            nc.scalar.activation(hid[:], psH[:], Relu, bias=Bias(3))
            st["hid"][u] = hid

        def seg_E(u):                      # out2 + q evac/DMA
            hid = st["hid"].pop(u)
            v = u % 4
            if v == 0:
                st["psQ"][u // 4] = psum.tile([128, 512], F32, tag="q", bufs=1, name=f"psQ{u // 4}")
            psQ = st["psQ"][u // 4]
            nc.tensor.matmul(psQ[32 * v:32 * (v + 1), :], Wt("W2"),
                             hid[:], start=True, stop=True,
                             tile_position=(0, 32 * v),
                             skip_group_check=True)
            if v == 3:
                b = u // 4
                qi = b % (QCH // 4)
                if qi == 0:
                    st["qsb"][b // (QCH // 4)] = pool.tile(
                        [128, (QCH // 4) * 512], BF16, tag="qsb", bufs=2,
                        name=f"qsb{b}")
                q_sb = st["qsb"][b // (QCH // 4)]
                del st["psQ"][b]
                nc.vector.tensor_tensor(q_sb[:, 512 * qi:512 * (qi + 1)],
                                        psQ[:], pen_t[:, b * 512:(b + 1) * 512],
                                        ALU.add)
                if qi == QCH // 4 - 1:
                    nc.sync.dma_start(q_d[u // QCH], q_sb[:])

        SEGS = [("A", 0), ("B", -2), ("Bp", -3), ("C", -5), ("Cp", -6),
                ("D", -8), ("E", -9)]

        def emit(name, u):
            if not 0 <= u < NSUP:
                return
            if name == "A":
                seg_A(u)
            elif name == "B":
                seg_round_a(u, 0)
            elif name == "Bp":
                seg_round_b(u, 0)
            elif name == "C":
                seg_round_a(u, 1)
            elif name == "Cp":
                seg_round_b(u, 1)
            elif name == "D":
                seg_D(u)
            elif name == "E":
                seg_E(u)

        for k in range(NSUP + 10):
            if k == 1 and not pen_started:
                nc.sync.dma_start(pen_t[:], pen_d[:])
                pen_started.append(True)
            for name, off in SEGS:
                emit(name, k + off)

    nc.compile()
    return nc


def _prep_host(obs, enc_w, enc_b, comm_w, comm_b, out_w1, out_b1, out_w2, out_b2,
               available_actions):
    bf16 = ml_dtypes.bfloat16
    f32 = np.float32

    def bd(w):  # block-diag duplicate [k,m] -> [2k, 2m]
        k, m = w.shape
        o = np.zeros((2 * k, 2 * m), f32)
        o[:k, :m] = w
        o[k:, m:] = w
        return o

    wall = np.zeros((128, WCOLS), f32)
    wall[:, WOFF["Wenc"]:WOFF["Wenc"] + 128] = bd(enc_w.astype(f32))
    wall[:, WOFF["W1"]:WOFF["W1"] + 128] = bd(out_w1.astype(f32))
    wall[:, WOFF["W2"]:WOFF["W2"] + 32] = bd(out_w2.astype(f32))
    wall[:, WOFF["idq"]:WOFF["idq"] + 128] = np.eye(128, dtype=f32)
    for r in range(NR):
        wh = comm_w[r][:H].astype(f32)
        wm = comm_w[r][H:].astype(f32) / (N - 1)
        wall[:, WOFF[f"Wself{r}"]:WOFF[f"Wself{r}"] + 128] = bd(wh - wm)
        wall[:, WOFF[f"Wsum{r}"]:WOFF[f"Wsum{r}"] + 128] = \
            np.tile(wm, (2, 2))
    wall = np.ascontiguousarray(wall).astype(bf16)

    ball = np.stack([np.concatenate([v, v]).astype(f32) for v in
                     (enc_b, comm_b[0], comm_b[1], out_b1)], axis=1)
    ball = np.ascontiguousarray(ball)

    rows = np.ascontiguousarray(obs.reshape(B * N, OBS))
    pen = np.where(available_actions.reshape(B * N, A) == 0,
                   f32(-1e10), out_b2.astype(f32)[None, :]).astype(bf16)

    in_maps = []
    for c in range(NCORES):
        ro = rows[c * RPC:(c + 1) * RPC]
        # [s, t, half, amod, f] -> [s, half, f, t, amod]
        opk = ro.reshape(NSUP, SAMP, 2, 16, OBS).transpose(0, 2, 4, 1, 3) \
                .reshape(NSUP // OCH, OCH, 128, SUP).transpose(0, 2, 1, 3) \
                .reshape(NSUP // OCH, 128, OCH * SUP).astype(bf16)
        pe = pen[c * RPC:(c + 1) * RPC]
        # [b, u, t, half, jj, act] -> [b, u, half, act, t, jj]; banks along cols
        ppk = pe.reshape(NSUP // 4, 4, SAMP, 2, 16, A) \
                .transpose(0, 1, 3, 5, 2, 4).reshape(NSUP // 4, 128, 512) \
                .transpose(1, 0, 2).reshape(128, (NSUP // 4) * 512)
        m = {"obs_pk": np.ascontiguousarray(opk),
             "pen_pk": np.ascontiguousarray(ppk),
             "Wall": wall, "Ball": ball}
        in_maps.append(m)
    return in_maps


def _unpack_output(results):
    qs = []
    for r in results:
        qpk = np.asarray(r["q_pk"]).astype(np.float32)
        # [nq, 128, QCH//4 * 512] -> [b, 128, 512]
        qpk = qpk.reshape(NSUP // QCH, 128, QCH // 4, 512).transpose(0, 2, 1, 3) \
                 .reshape(NSUP // 4, 128, 512)
        # [b, u, half, act, t, jj] -> [b, u, t, half, jj, act]
        q = qpk.reshape(NSUP // 4, 4, 2, A, SAMP, 16) \
               .transpose(0, 1, 4, 2, 5, 3).reshape(RPC, A)
        qs.append(q)
    return np.concatenate(qs, axis=0).reshape(B, N, A)


def run_on_device(in_maps, trace=False):
    from concourse.bass_utils import run_bass_kernel_spmd

    if "nc" not in _cache:
        _cache["nc"] = _build_device_program()
    return run_bass_kernel_spmd(_cache["nc"], in_maps,
                                core_ids=list(range(NCORES)), trace=trace)


def kernel(obs, enc_w, enc_b, comm_w, comm_b, out_w1, out_b1, out_w2, out_b2,
           available_actions):
    args = [np.asarray(x) for x in
            (obs, enc_w, enc_b, comm_w, comm_b, out_w1, out_b1, out_w2, out_b2,
             available_actions)]
    in_maps = _prep_host(*args)
    res = run_on_device(in_maps)
    return _unpack_output(res.results)


# revision 5
# speedup vs baseline: 1.0181x; 1.0181x over previous
"""CommNet forward kernel v2 for 8 Trainium2 NeuronCores.

Layout: feature-major, TWO rows (agents) per PE column via K=128
block-diagonal weights.  A unit is [128, 512]: partitions = 2 halves x
64 features (row half A = agents 0-15, half B = agents 16-31), columns
= 32 samples x 16 (16 adjacent columns = one sample's 32 agents split
across the halves).  Every dense layer is a K=128, M=128 block-diag
matmul charging 512 output columns per 1024 rows — half the per-row
cost of a K=64 layout.  512-col units keep each stage PSUM tile to one
bank (PSUM: 8 banks is the pipelining constraint).

DMA discipline: the cost model holds the issuing sequencer for the
whole descriptor-gen + transfer + completion-sem chain (~3 us per
dma_start), so everything is batched: one weights DMA, one biases DMA,
one pen DMA, obs in 8-unit chunks, q out every 8 units.

Per unit (512 cols, 32 samples):
  psE  = Wenc_bd  @ obs
  h0   = relu(psE + be)            [Act]
  round r in {0,1}:
    S2  = groupwise tensor_reduce(h) [DVE] -> [128, 32] bf16 in SBUF
    psR = Wself_bd[r] @ h  (+)  Wsum_vs[r] @ S2-broadcast (K=128 folds
          the cross-half agent sum into the matmul)
    h   = relu(psR + b_r)          [Pool]
  psH  = W1_bd @ h;  hid = relu(psH + bh)   [Act]
  out2: [32,512] into q-bank partition range 32*(u%4)
  every 4 units: q = psQ + pen (mask+bias folded host-side) [DVE]
"""

import contextlib
import sys

import numpy as np

sys.path.insert(0, "/opt/trn_rl_repo")

import ml_dtypes  # noqa: E402

B, N, OBS, H, A, NR = 16384, 32, 64, 64, 16, 2
NCORES = 8
RPC = B * N // NCORES      # rows per core = 65536

SUP = 512                  # columns per unit (= 1024 rows, 32 samples)
NSUP = RPC // (2 * SUP)    # 64 units
SAMP = SUP // 16           # samples per unit = 32
OCH = 8                    # units per obs DMA chunk
QCH = 8                    # units per q DMA chunk (2 q banks)

# weight buffer column offsets
WOFF = {"Wenc": 0, "Wself0": 128, "Wself1": 256, "W1": 384,
        "Wsum0": 512, "Wsum1": 640, "W2": 768, "idq": 800}
WCOLS = 928

_cache = {}


def _build_device_program():
    import concourse.bacc as bacc
    import concourse.mybir as mybir
    from concourse import tile

    F32 = mybir.dt.float32
    BF16 = mybir.dt.bfloat16
    Relu = mybir.ActivationFunctionType.Relu
    Copy = mybir.ActivationFunctionType.Copy
    ALU = mybir.AluOpType
    AXIS = mybir.AxisListType

    nc = bacc.Bacc("TRN2", target_bir_lowering=False, debug=False)

    obs_d = nc.dram_tensor("obs_pk", [NSUP // OCH, 128, OCH * SUP], BF16,
                           kind="ExternalInput")
    pen_d = nc.dram_tensor("pen_pk", [128, (NSUP // 4) * 512], BF16,
                           kind="ExternalInput")
    q_d = nc.dram_tensor("q_pk", [NSUP // QCH, 128, (QCH // 4) * 512], BF16,
                         kind="ExternalOutput")
    wall_d = nc.dram_tensor("Wall", [128, WCOLS], BF16, kind="ExternalInput")
    ball_d = nc.dram_tensor("Ball", [128, 4], F32, kind="ExternalInput")

    with tile.TileContext(nc) as tc, contextlib.ExitStack() as ctx:
        wp = ctx.enter_context(tc.tile_pool(name="w", bufs=1))
        pool = ctx.enter_context(tc.tile_pool(name="p", bufs=2))
        psum = ctx.enter_context(tc.tile_pool(name="ps", bufs=1, space="PSUM"))

        wall = wp.tile([128, WCOLS], BF16, tag="wall", name="wall")
        nc.sync.dma_start(wall[:], wall_d[:])
        ball = wp.tile([128, 4], F32, tag="ball", name="ball")
        nc.sync.dma_start(ball[:], ball_d[:])
        pen_t = wp.tile([128, (NSUP // 4) * 512], BF16, tag="pen", name="pen")
        pen_started = []

        def Wt(n):
            return wall[:, WOFF[n]:WOFF[n] + (32 if n == "W2" else 128)]

        def Bias(i):
            return ball[:, i:i + 1]

        # ---- software-pipelined emission across units -----------------
        # Segments per unit u (PE work + attached evacs), emitted with
        # unit-offsets so the PE stream interleaves ~4 units and every
        # matmul's inputs are evacuated well before the PE reaches it.
        st = {"obs": {}, "h0": {}, "h1": {}, "h2": {}, "hid": {},
              "S2": {}, "psR": {}, "psQ": {}, "qsb": {}}

        def seg_A(u):                      # enc
            if u % OCH == 0:
                obs_t = pool.tile([128, OCH * SUP], BF16, tag="obs", bufs=2, name=f"obs{u}")
                if u == 0:
                    # split the first chunk so enc(0) starts sooner
                    hw = OCH * SUP // 4
                    nc.sync.dma_start(obs_t[:, 0:hw], obs_d[0][:, 0:hw])
                    nc.sync.dma_start(obs_t[:, hw:], obs_d[0][:, hw:])
                else:
                    nc.sync.dma_start(obs_t[:], obs_d[u // OCH])
                st["obs"][u // OCH] = obs_t
            obs_t = st["obs"][u // OCH]
            osl = obs_t[:, (u % OCH) * SUP:(u % OCH + 1) * SUP]
            psE = psum.tile([128, SUP], F32, tag="stg", bufs=7, name=f"psE{u}")
            nc.tensor.matmul(psE[:], Wt("Wenc"), osl, start=True, stop=True)
            h = pool.tile([128, SUP], BF16, tag="h", bufs=24, name=f"h0_{u}")
            nc.scalar.activation(h[:], psE[:], Relu, bias=Bias(0))
            st["h0"][u] = h

        def seg_round_a(u, r):             # S-reduce + Wself
            h = st["h0" if r == 0 else "h1"][u]
            S2 = pool.tile([128, SAMP], BF16, tag="S2", bufs=6, name=f"S2_{u}_{r}")
            if r == 0:
                # Pool (gpsimd) cannot touch PSUM, but it can take this
                # all-SBUF pairwise-add tree off the DVE/Act critical path.
                tmp = pool.tile([128, 448], BF16, tag="rt", bufs=3,
                                name=f"rt{u}_{r}")
                src_ap, width = h, 512
                off = 0
                outs = [(tmp, 0, 256), (tmp, 256, 128), (tmp, 384, 64),
                        (S2, 0, 32)]
                prev = h[:]
                prev_off = 0
                for (dst, doff, w) in outs:
                    i3 = prev.rearrange("p (t e) -> p t e", e=2)
                    o3 = dst[:, doff:doff + w].rearrange("p (t o) -> p t o", o=1)
                    nc.gpsimd.tensor_tensor(o3, i3[:, :, 0:1], i3[:, :, 1:2],
                                            ALU.add)
                    prev = dst[:, doff:doff + w]
            else:
                with nc.allow_low_precision("bf16 sums feed 1/31-scaled msg"):
                    nc.vector.tensor_reduce(
                        S2[:], h[:].rearrange("p (t n) -> p t n", n=16),
                        AXIS.X, ALU.add)
            st["S2"][(u, r)] = S2
            psR = psum.tile([128, SUP], F32, tag="stg", bufs=7, name=f"psR{u}_{r}")
            nc.tensor.matmul(psR[:], Wt(f"Wself{r}"), h[:],
                             start=True, stop=False)
            st["psR"][(u, r)] = psR

        def seg_round_b(u, r):             # Wsum-broadcast + evac
            psR = st["psR"].pop((u, r))
            S2 = st["S2"].pop((u, r))
            rhs = S2[:].unsqueeze(2).broadcast_to([128, SAMP, 16])
            nc.tensor.matmul(
                psR[:].rearrange("p (t n) -> p t n", n=16),
                Wt(f"Wsum{r}"), rhs, start=False, stop=True)
            h = pool.tile([128, SUP], BF16, tag="h", bufs=24, name=f"h{1 + r}_{u}")
            if r == 1 and u % 3 != 2:
                nc.scalar.activation(h[:], psR[:], Relu, bias=Bias(2))
            else:
                nc.vector.tensor_scalar(h[:], psR[:], Bias(1 + r), 0.0,
                                        ALU.add, ALU.max)
            st["h1" if r == 0 else "h2"][u] = h

        def seg_D(u):                      # out1
            h = st["h2"].pop(u)
            psH = psum.tile([128, SUP], F32, tag="stg", bufs=7, name=f"psH{u}")
            nc.tensor.matmul(psH[:], Wt("W1"), h[:], start=True, stop=True)
            hid = pool.tile([128, SUP], BF16, tag="h", bufs=24, name=f"hid{u}")
            nc.scalar.activation(hid[:], psH[:], Relu, bias=Bias(3))
            st["hid"][u] = hid

        def seg_E(u):                      # out2 + q evac/DMA
            hid = st["hid"].pop(u)
            v = u % 4
            if v == 0:
                st["psQ"][u // 4] = psum.tile([128, 512], F32, tag="q", bufs=1, name=f"psQ{u // 4}")
            psQ = st["psQ"][u // 4]
            nc.tensor.matmul(psQ[32 * v:32 * (v + 1), :], Wt("W2"),
                             hid[:], start=True, stop=True,
                             tile_position=(0, 32 * v),
                             skip_group_check=True)
            if v == 3:
                b = u // 4
                qi = b % (QCH // 4)
                if qi == 0:
                    st["qsb"][b // (QCH // 4)] = pool.tile(
                        [128, (QCH // 4) * 512], BF16, tag="qsb", bufs=2,
                        name=f"qsb{b}")
                q_sb = st["qsb"][b // (QCH // 4)]
                del st["psQ"][b]
                nc.vector.tensor_tensor(q_sb[:, 512 * qi:512 * (qi + 1)],
                                        psQ[:], pen_t[:, b * 512:(b + 1) * 512],
                                        ALU.add)
                if qi == QCH // 4 - 1:
                    nc.sync.dma_start(q_d[u // QCH], q_sb[:])

        SEGS = [("A", 0), ("B", -2), ("Bp", -3), ("C", -5), ("Cp", -6),
                ("D", -8), ("E", -9)]

        def emit(name, u):
            if not 0 <= u < NSUP:
                return
            if name == "A":
                seg_A(u)
            elif name == "B":
                seg_round_a(u, 0)
            elif name == "Bp":
                seg_round_b(u, 0)
            elif name == "C":
                seg_round_a(u, 1)
            elif name == "Cp":
                seg_round_b(u, 1)
            elif name == "D":
                seg_D(u)
            elif name == "E":
                seg_E(u)

        for k in range(NSUP + 10):
            if k == 1 and not pen_started:
                nc.sync.dma_start(pen_t[:], pen_d[:])
                pen_started.append(True)
            for name, off in SEGS:
                emit(name, k + off)

    nc.compile()
    return nc


def _prep_host(obs, enc_w, enc_b, comm_w, comm_b, out_w1, out_b1, out_w2, out_b2,
               available_actions):
    bf16 = ml_dtypes.bfloat16
    f32 = np.float32

    def bd(w):  # block-diag duplicate [k,m] -> [2k, 2m]
        k, m = w.shape
        o = np.zeros((2 * k, 2 * m), f32)
        o[:k, :m] = w
        o[k:, m:] = w
        return o

    wall = np.zeros((128, WCOLS), f32)
    wall[:, WOFF["Wenc"]:WOFF["Wenc"] + 128] = bd(enc_w.astype(f32))
    wall[:, WOFF["W1"]:WOFF["W1"] + 128] = bd(out_w1.astype(f32))
    wall[:, WOFF["W2"]:WOFF["W2"] + 32] = bd(out_w2.astype(f32))
    wall[:, WOFF["idq"]:WOFF["idq"] + 128] = np.eye(128, dtype=f32)
    for r in range(NR):
        wh = comm_w[r][:H].astype(f32)
        wm = comm_w[r][H:].astype(f32) / (N - 1)
        wall[:, WOFF[f"Wself{r}"]:WOFF[f"Wself{r}"] + 128] = bd(wh - wm)
        wall[:, WOFF[f"Wsum{r}"]:WOFF[f"Wsum{r}"] + 128] = \
            np.tile(wm, (2, 2))
    wall = np.ascontiguousarray(wall).astype(bf16)

    ball = np.stack([np.concatenate([v, v]).astype(f32) for v in
                     (enc_b, comm_b[0], comm_b[1], out_b1)], axis=1)
    ball = np.ascontiguousarray(ball)

    rows = np.ascontiguousarray(obs.reshape(B * N, OBS))
    pen = np.where(available_actions.reshape(B * N, A) == 0,
                   f32(-1e10), out_b2.astype(f32)[None, :]).astype(bf16)

    in_maps = []
    for c in range(NCORES):
        ro = rows[c * RPC:(c + 1) * RPC]
        # [s, t, half, amod, f] -> [s, half, f, t, amod]
        opk = ro.reshape(NSUP, SAMP, 2, 16, OBS).transpose(0, 2, 4, 1, 3) \
                .reshape(NSUP // OCH, OCH, 128, SUP).transpose(0, 2, 1, 3) \
                .reshape(NSUP // OCH, 128, OCH * SUP).astype(bf16)
        pe = pen[c * RPC:(c + 1) * RPC]
        # [b, u, t, half, jj, act] -> [b, u, half, act, t, jj]; banks along cols
        ppk = pe.reshape(NSUP // 4, 4, SAMP, 2, 16, A) \
                .transpose(0, 1, 3, 5, 2, 4).reshape(NSUP // 4, 128, 512) \
                .transpose(1, 0, 2).reshape(128, (NSUP // 4) * 512)
        m = {"obs_pk": np.ascontiguousarray(opk),
             "pen_pk": np.ascontiguousarray(ppk),
             "Wall": wall, "Ball": ball}
        in_maps.append(m)
    return in_maps


def _unpack_output(results):
    qs = []
    for r in results:
        qpk = np.asarray(r["q_pk"]).astype(np.float32)
        # [nq, 128, QCH//4 * 512] -> [b, 128, 512]
        qpk = qpk.reshape(NSUP // QCH, 128, QCH // 4, 512).transpose(0, 2, 1, 3) \
                 .reshape(NSUP // 4, 128, 512)
        # [b, u, half, act, t, jj] -> [b, u, t, half, jj, act]
        q = qpk.reshape(NSUP // 4, 4, 2, A, SAMP, 16) \
               .transpose(0, 1, 4, 2, 5, 3).reshape(RPC, A)
        qs.append(q)
    return np.concatenate(qs, axis=0).reshape(B, N, A)


def run_on_device(in_maps, trace=False):
    from concourse.bass_utils import run_bass_kernel_spmd

    if "nc" not in _cache:
        _cache["nc"] = _build_device_program()
    return run_bass_kernel_spmd(_cache["nc"], in_maps,
                                core_ids=list(range(NCORES)), trace=trace)


def kernel(obs, enc_w, enc_b, comm_w, comm_b, out_w1, out_b1, out_w2, out_b2,
           available_actions):
    args = [np.asarray(x) for x in
            (obs, enc_w, enc_b, comm_w, comm_b, out_w1, out_b1, out_w2, out_b2,
             available_actions)]
    in_maps = _prep_host(*args)
    res = run_on_device(in_maps)
    return _unpack_output(res.results)


# revision 6
# speedup vs baseline: 1.0215x; 1.0034x over previous
"""CommNet forward kernel v2 for 8 Trainium2 NeuronCores.

Layout: feature-major, TWO rows (agents) per PE column via K=128
block-diagonal weights.  A unit is [128, 512]: partitions = 2 halves x
64 features (row half A = agents 0-15, half B = agents 16-31), columns
= 32 samples x 16 (16 adjacent columns = one sample's 32 agents split
across the halves).  Every dense layer is a K=128, M=128 block-diag
matmul charging 512 output columns per 1024 rows — half the per-row
cost of a K=64 layout.  512-col units keep each stage PSUM tile to one
bank (PSUM: 8 banks is the pipelining constraint).

DMA discipline: the cost model holds the issuing sequencer for the
whole descriptor-gen + transfer + completion-sem chain (~3 us per
dma_start), so everything is batched: one weights DMA, one biases DMA,
one pen DMA, obs in 8-unit chunks, q out every 8 units.

Per unit (512 cols, 32 samples):
  psE  = Wenc_bd  @ obs
  h0   = relu(psE + be)            [Act]
  round r in {0,1}:
    S2  = groupwise tensor_reduce(h) [DVE] -> [128, 32] bf16 in SBUF
    psR = Wself_bd[r] @ h  (+)  Wsum_vs[r] @ S2-broadcast (K=128 folds
          the cross-half agent sum into the matmul)
    h   = relu(psR + b_r)          [Pool]
  psH  = W1_bd @ h;  hid = relu(psH + bh)   [Act]
  out2: [32,512] into q-bank partition range 32*(u%4)
  every 4 units: q = psQ + pen (mask+bias folded host-side) [DVE]
"""

import contextlib
import sys

import numpy as np

sys.path.insert(0, "/opt/trn_rl_repo")

import ml_dtypes  # noqa: E402

B, N, OBS, H, A, NR = 16384, 32, 64, 64, 16, 2
NCORES = 8
RPC = B * N // NCORES      # rows per core = 65536

SUP = 512                  # columns per unit (= 1024 rows, 32 samples)
NSUP = RPC // (2 * SUP)    # 64 units
SAMP = SUP // 16           # samples per unit = 32
OCH = 8                    # units per obs DMA chunk
QCH = 8                    # units per q DMA chunk (2 q banks)

# weight buffer column offsets
WOFF = {"Wenc": 0, "Wself0": 128, "Wself1": 256, "W1": 384,
        "Wsum0": 512, "Wsum1": 640, "W2": 768, "idq": 800}
WCOLS = 928

_cache = {}


def _build_device_program():
    import concourse.bacc as bacc
    import concourse.mybir as mybir
    from concourse import tile

    F32 = mybir.dt.float32
    BF16 = mybir.dt.bfloat16
    Relu = mybir.ActivationFunctionType.Relu
    Copy = mybir.ActivationFunctionType.Copy
    ALU = mybir.AluOpType
    AXIS = mybir.AxisListType

    nc = bacc.Bacc("TRN2", target_bir_lowering=False, debug=False)

    obs_d = nc.dram_tensor("obs_pk", [NSUP // OCH, 128, OCH * SUP], BF16,
                           kind="ExternalInput")
    pen_d = nc.dram_tensor("pen_pk", [128, (NSUP // 4) * 512], BF16,
                           kind="ExternalInput")
    q_d = nc.dram_tensor("q_pk", [NSUP // QCH, 128, (QCH // 4) * 512], BF16,
                         kind="ExternalOutput")
    wall_d = nc.dram_tensor("Wall", [128, WCOLS], BF16, kind="ExternalInput")
    ball_d = nc.dram_tensor("Ball", [128, 4], F32, kind="ExternalInput")

    with tile.TileContext(nc) as tc, contextlib.ExitStack() as ctx:
        wp = ctx.enter_context(tc.tile_pool(name="w", bufs=1))
        pool = ctx.enter_context(tc.tile_pool(name="p", bufs=2))
        psum = ctx.enter_context(tc.tile_pool(name="ps", bufs=1, space="PSUM"))

        wall = wp.tile([128, WCOLS], BF16, tag="wall", name="wall")
        nc.sync.dma_start(wall[:], wall_d[:])
        ball = wp.tile([128, 4], F32, tag="ball", name="ball")
        nc.sync.dma_start(ball[:], ball_d[:])
        pen_t = wp.tile([128, (NSUP // 4) * 512], BF16, tag="pen", name="pen")
        pen_started = []

        def Wt(n):
            return wall[:, WOFF[n]:WOFF[n] + (32 if n == "W2" else 128)]

        def Bias(i):
            return ball[:, i:i + 1]

        # ---- software-pipelined emission across units -----------------
        # Segments per unit u (PE work + attached evacs), emitted with
        # unit-offsets so the PE stream interleaves ~4 units and every
        # matmul's inputs are evacuated well before the PE reaches it.
        st = {"obs": {}, "h0": {}, "h1": {}, "h2": {}, "hid": {},
              "S2": {}, "psR": {}, "psQ": {}, "qsb": {}}

        def seg_A(u):                      # enc
            if u % OCH == 0:
                obs_t = pool.tile([128, OCH * SUP], BF16, tag="obs", bufs=2, name=f"obs{u}")
                if u == 0:
                    # split the first chunk so enc(0) starts sooner
                    hw = OCH * SUP // 4
                    nc.sync.dma_start(obs_t[:, 0:hw], obs_d[0][:, 0:hw])
                    nc.sync.dma_start(obs_t[:, hw:], obs_d[0][:, hw:])
                else:
                    nc.sync.dma_start(obs_t[:], obs_d[u // OCH])
                st["obs"][u // OCH] = obs_t
            obs_t = st["obs"][u // OCH]
            osl = obs_t[:, (u % OCH) * SUP:(u % OCH + 1) * SUP]
            psE = psum.tile([128, SUP], F32, tag="stg", bufs=7, name=f"psE{u}")
            nc.tensor.matmul(psE[:], Wt("Wenc"), osl, start=True, stop=True)
            h = pool.tile([128, SUP], BF16, tag="h", bufs=24, name=f"h0_{u}")
            nc.scalar.activation(h[:], psE[:], Relu, bias=Bias(0))
            st["h0"][u] = h

        def seg_round_a(u, r):             # S-reduce + Wself
            h = st["h0" if r == 0 else "h1"][u]
            S2 = pool.tile([128, SAMP], BF16, tag="S2", bufs=6, name=f"S2_{u}_{r}")
            if r == 0:
                # Pool (gpsimd) cannot touch PSUM, but it can take this
                # all-SBUF pairwise-add tree off the DVE/Act critical path.
                tmp = pool.tile([128, 448], BF16, tag="rt", bufs=3,
                                name=f"rt{u}_{r}")
                src_ap, width = h, 512
                off = 0
                outs = [(tmp, 0, 256), (tmp, 256, 128), (tmp, 384, 64),
                        (S2, 0, 32)]
                prev = h[:]
                prev_off = 0
                for (dst, doff, w) in outs:
                    i3 = prev.rearrange("p (t e) -> p t e", e=2)
                    o3 = dst[:, doff:doff + w].rearrange("p (t o) -> p t o", o=1)
                    nc.gpsimd.tensor_tensor(o3, i3[:, :, 0:1], i3[:, :, 1:2],
                                            ALU.add)
                    prev = dst[:, doff:doff + w]
            else:
                with nc.allow_low_precision("bf16 sums feed 1/31-scaled msg"):
                    nc.vector.tensor_reduce(
                        S2[:], h[:].rearrange("p (t n) -> p t n", n=16),
                        AXIS.X, ALU.add)
            st["S2"][(u, r)] = S2
            psR = psum.tile([128, SUP], F32, tag="stg", bufs=7, name=f"psR{u}_{r}")
            nc.tensor.matmul(psR[:], Wt(f"Wself{r}"), h[:],
                             start=True, stop=False)
            st["psR"][(u, r)] = psR

        def seg_round_b(u, r):             # Wsum-broadcast + evac
            psR = st["psR"].pop((u, r))
            S2 = st["S2"].pop((u, r))
            rhs = S2[:].unsqueeze(2).broadcast_to([128, SAMP, 16])
            nc.tensor.matmul(
                psR[:].rearrange("p (t n) -> p t n", n=16),
                Wt(f"Wsum{r}"), rhs, start=False, stop=True)
            h = pool.tile([128, SUP], BF16, tag="h", bufs=24, name=f"h{1 + r}_{u}")
            if r == 1 and u % 3 != 2:
                nc.scalar.activation(h[:], psR[:], Relu, bias=Bias(2))
            else:
                nc.vector.tensor_scalar(h[:], psR[:], Bias(1 + r), 0.0,
                                        ALU.add, ALU.max)
            st["h1" if r == 0 else "h2"][u] = h

        def seg_D(u):                      # out1
            h = st["h2"].pop(u)
            psH = psum.tile([128, SUP], F32, tag="stg", bufs=7, name=f"psH{u}")
            nc.tensor.matmul(psH[:], Wt("W1"), h[:], start=True, stop=True)
            hid = pool.tile([128, SUP], BF16, tag="h", bufs=24, name=f"hid{u}")
            nc.scalar.activation(hid[:], psH[:], Relu, bias=Bias(3))
            st["hid"][u] = hid

        def seg_E(u):                      # out2 + q evac/DMA
            hid = st["hid"].pop(u)
            v = u % 4
            if v == 0:
                st["psQ"][u // 4] = psum.tile([128, 512], F32, tag="q", bufs=1, name=f"psQ{u // 4}")
            psQ = st["psQ"][u // 4]
            nc.tensor.matmul(psQ[32 * v:32 * (v + 1), :], Wt("W2"),
                             hid[:], start=True, stop=True,
                             tile_position=(0, 32 * v),
                             skip_group_check=True)
            if v == 3:
                b = u // 4
                qi = b % (QCH // 4)
                if qi == 0:
                    st["qsb"][b // (QCH // 4)] = pool.tile(
                        [128, (QCH // 4) * 512], BF16, tag="qsb", bufs=2,
                        name=f"qsb{b}")
                q_sb = st["qsb"][b // (QCH // 4)]
                del st["psQ"][b]
                nc.vector.tensor_tensor(q_sb[:, 512 * qi:512 * (qi + 1)],
                                        psQ[:], pen_t[:, b * 512:(b + 1) * 512],
                                        ALU.add)
                ch = u // QCH
                if ch == NSUP // QCH - 1:
                    # final chunk: ship each bank as soon as its TT lands so
                    # only the last 512 cols trail the last compute
                    nc.sync.dma_start(q_d[ch][:, 512 * qi:512 * (qi + 1)],
                                      q_sb[:, 512 * qi:512 * (qi + 1)])
                elif qi == QCH // 4 - 1:
                    nc.sync.dma_start(q_d[ch], q_sb[:])

        SEGS = [("A", 0), ("B", -2), ("Bp", -3), ("C", -5), ("Cp", -6),
                ("D", -8), ("E", -9)]

        def emit(name, u):
            if not 0 <= u < NSUP:
                return
            if name == "A":
                seg_A(u)
            elif name == "B":
                seg_round_a(u, 0)
            elif name == "Bp":
                seg_round_b(u, 0)
            elif name == "C":
                seg_round_a(u, 1)
            elif name == "Cp":
                seg_round_b(u, 1)
            elif name == "D":
                seg_D(u)
            elif name == "E":
                seg_E(u)

        for k in range(NSUP + 10):
            if k == 1 and not pen_started:
                nc.sync.dma_start(pen_t[:], pen_d[:])
                pen_started.append(True)
            for name, off in SEGS:
                emit(name, k + off)

    nc.compile()
    return nc


def _prep_host(obs, enc_w, enc_b, comm_w, comm_b, out_w1, out_b1, out_w2, out_b2,
               available_actions):
    bf16 = ml_dtypes.bfloat16
    f32 = np.float32

    def bd(w):  # block-diag duplicate [k,m] -> [2k, 2m]
        k, m = w.shape
        o = np.zeros((2 * k, 2 * m), f32)
        o[:k, :m] = w
        o[k:, m:] = w
        return o

    wall = np.zeros((128, WCOLS), f32)
    wall[:, WOFF["Wenc"]:WOFF["Wenc"] + 128] = bd(enc_w.astype(f32))
    wall[:, WOFF["W1"]:WOFF["W1"] + 128] = bd(out_w1.astype(f32))
    wall[:, WOFF["W2"]:WOFF["W2"] + 32] = bd(out_w2.astype(f32))
    wall[:, WOFF["idq"]:WOFF["idq"] + 128] = np.eye(128, dtype=f32)
    for r in range(NR):
        wh = comm_w[r][:H].astype(f32)
        wm = comm_w[r][H:].astype(f32) / (N - 1)
        wall[:, WOFF[f"Wself{r}"]:WOFF[f"Wself{r}"] + 128] = bd(wh - wm)
        wall[:, WOFF[f"Wsum{r}"]:WOFF[f"Wsum{r}"] + 128] = \
            np.tile(wm, (2, 2))
    wall = np.ascontiguousarray(wall).astype(bf16)

    ball = np.stack([np.concatenate([v, v]).astype(f32) for v in
                     (enc_b, comm_b[0], comm_b[1], out_b1)], axis=1)
    ball = np.ascontiguousarray(ball)

    rows = np.ascontiguousarray(obs.reshape(B * N, OBS))
    pen = np.where(available_actions.reshape(B * N, A) == 0,
                   f32(-1e10), out_b2.astype(f32)[None, :]).astype(bf16)

    in_maps = []
    for c in range(NCORES):
        ro = rows[c * RPC:(c + 1) * RPC]
        # [s, t, half, amod, f] -> [s, half, f, t, amod]
        opk = ro.reshape(NSUP, SAMP, 2, 16, OBS).transpose(0, 2, 4, 1, 3) \
                .reshape(NSUP // OCH, OCH, 128, SUP).transpose(0, 2, 1, 3) \
                .reshape(NSUP // OCH, 128, OCH * SUP).astype(bf16)
        pe = pen[c * RPC:(c + 1) * RPC]
        # [b, u, t, half, jj, act] -> [b, u, half, act, t, jj]; banks along cols
        ppk = pe.reshape(NSUP // 4, 4, SAMP, 2, 16, A) \
                .transpose(0, 1, 3, 5, 2, 4).reshape(NSUP // 4, 128, 512) \
                .transpose(1, 0, 2).reshape(128, (NSUP // 4) * 512)
        m = {"obs_pk": np.ascontiguousarray(opk),
             "pen_pk": np.ascontiguousarray(ppk),
             "Wall": wall, "Ball": ball}
        in_maps.append(m)
    return in_maps


def _unpack_output(results):
    qs = []
    for r in results:
        qpk = np.asarray(r["q_pk"]).astype(np.float32)
        # [nq, 128, QCH//4 * 512] -> [b, 128, 512]
        qpk = qpk.reshape(NSUP // QCH, 128, QCH // 4, 512).transpose(0, 2, 1, 3) \
                 .reshape(NSUP // 4, 128, 512)
        # [b, u, half, act, t, jj] -> [b, u, t, half, jj, act]
        q = qpk.reshape(NSUP // 4, 4, 2, A, SAMP, 16) \
               .transpose(0, 1, 4, 2, 5, 3).reshape(RPC, A)
        qs.append(q)
    return np.concatenate(qs, axis=0).reshape(B, N, A)


def run_on_device(in_maps, trace=False):
    from concourse.bass_utils import run_bass_kernel_spmd

    if "nc" not in _cache:
        _cache["nc"] = _build_device_program()
    return run_bass_kernel_spmd(_cache["nc"], in_maps,
                                core_ids=list(range(NCORES)), trace=trace)


def kernel(obs, enc_w, enc_b, comm_w, comm_b, out_w1, out_b1, out_w2, out_b2,
           available_actions):
    args = [np.asarray(x) for x in
            (obs, enc_w, enc_b, comm_w, comm_b, out_w1, out_b1, out_w2, out_b2,
             available_actions)]
    in_maps = _prep_host(*args)
    res = run_on_device(in_maps)
    return _unpack_output(res.results)
